# revision 15
# baseline (speedup 1.0000x reference)
"""Hawk (RG-LRU) block kernel for Trainium2, SPMD over 8 NeuronCores.

Sharding: tokens. Core k handles batch b=k//2, half h=k%2 (2048 tokens).
Weights replicated, host-transposed, bf16 (full PE rate, half the HBM
traffic). Two fused passes over 4 token tiles of 512:

  pass 1: in-proj -> causal conv (DVE, bf16) -> gates matmul ->
          tanh/exp/ln activation chain (sigmoid via tanh so tanh+exp
          share one act-func table; beta via ln+exp) -> u ->
          h-scan + alpha-prefix-scan (DVE, fp32 state, bf16 out);
          h,p spill bf16 via the idle GPSIMD DMA queue.
  carry:  pairwise 4KB AllReduce moves the cross-half scan carry.
  pass 2: gate-proj + gelu, carry correction, out-proj one tile behind
          so PE never waits on the vector chain; out stored bf16.

alpha^2 runs on the (otherwise idle) GPSIMD engine. DMAs are batched
(one per tile per stream) and spread over SP/Act/Pool queues to avoid
sequencer head-of-line blocking.
"""
import sys

sys.path.insert(0, "/opt/trn_rl_repo")

import numpy as np
import ml_dtypes
from contextlib import ExitStack

import concourse.bass as bass
import concourse.tile as tile
import concourse.bacc as bacc
from concourse import mybir
from concourse.bass_utils import run_bass_kernel_spmd

F32 = mybir.dt.float32
BF16 = mybir.dt.bfloat16
AF = mybir.ActivationFunctionType
OP = mybir.AluOpType

B, T, DIM = 4, 4096, 1024
E = 1024
KC = 4
N_CORES = 8
T_LOC = T // 2
TT = 512
NTT = T_LOC // TT   # 4
NE = E // 128       # 8
NK = DIM // 128     # 8

ALPHA2_ON_POOL = True


def _build_kernel(profile_mode=False):
    nc = bacc.Bacc("TRN2", target_bir_lowering=False, debug=False,
                   num_devices=1 if profile_mode else N_CORES)

    xT = nc.dram_tensor("xT", [DIM, T_LOC], BF16, kind="ExternalInput")
    xa_halo = nc.dram_tensor("xa_halo", [E, KC - 1], BF16, kind="ExternalInput")
    w_in_g = nc.dram_tensor("w_in_g", [DIM, E], BF16, kind="ExternalInput")
    w_in_x = nc.dram_tensor("w_in_x", [DIM, E], BF16, kind="ExternalInput")
    w_gates = nc.dram_tensor("w_gates", [E, 2 * E], BF16, kind="ExternalInput")
    w_out = nc.dram_tensor("w_out", [E, DIM], BF16, kind="ExternalInput")
    wc = nc.dram_tensor("wc", [E, KC], F32, kind="ExternalInput")
    b_conv = nc.dram_tensor("b_conv", [E, 1], F32, kind="ExternalInput")
    neg_ch = nc.dram_tensor("neg_ch", [E, 1], F32, kind="ExternalInput")
    b_fh = nc.dram_tensor("b_fh", [E, 1], F32, kind="ExternalInput")
    b_ih = nc.dram_tensor("b_ih", [E, 1], F32, kind="ExternalInput")
    mask_c = nc.dram_tensor("mask_c", [128, 1], F32, kind="ExternalInput")
    mask_u = nc.dram_tensor("mask_u", [128, 1], F32, kind="ExternalInput")
    out = nc.dram_tensor("out", [T_LOC, DIM], BF16, kind="ExternalOutput")

    with tile.TileContext(nc) as tc, ExitStack() as ctx:
        _body(ctx, tc, nc, profile_mode=profile_mode,
              xT=xT, xa_halo=xa_halo, w_in_g=w_in_g, w_in_x=w_in_x,
              w_gates=w_gates, w_out=w_out, wc=wc, b_conv=b_conv,
              neg_ch=neg_ch, b_fh=b_fh, b_ih=b_ih,
              mask_c=mask_c, mask_u=mask_u, out=out)
    nc.compile()
    return nc


def _body(ctx, tc, nc, *, xT, xa_halo, w_in_g, w_in_x, w_gates, w_out, wc,
          b_conv, neg_ch, b_fh, b_ih, mask_c, mask_u, out,
          profile_mode=False):
    consts = ctx.enter_context(tc.tile_pool(name="consts", bufs=1))
    ps = ctx.enter_context(tc.tile_pool(name="ps", bufs=8, space="PSUM"))
    dram = ctx.enter_context(tc.tile_pool(name="dram", bufs=1, space="DRAM"))
    wpool = ctx.enter_context(tc.tile_pool(name="weights", bufs=1, side="right"))
    xs = ctx.enter_context(tc.tile_pool(name="xs", bufs=3))
    xap = ctx.enter_context(tc.tile_pool(name="xap", bufs=2))
    xcp = ctx.enter_context(tc.tile_pool(name="xcp", bufs=2))
    sfp = ctx.enter_context(tc.tile_pool(name="sfp", bufs=9))
    sip = ctx.enter_context(tc.tile_pool(name="sip", bufs=9))
    alp = ctx.enter_context(tc.tile_pool(name="alp", bufs=3))
    a2p = ctx.enter_context(tc.tile_pool(name="a2p", bufs=3))
    bep = ctx.enter_context(tc.tile_pool(name="bep", bufs=3))
    bsp = ctx.enter_context(tc.tile_pool(name="bsp", bufs=2))
    up = ctx.enter_context(tc.tile_pool(name="up", bufs=2))
    spl = ctx.enter_context(tc.tile_pool(name="spl", bufs=2))
    hvp = ctx.enter_context(tc.tile_pool(name="hvp", bufs=2))
    osbp = ctx.enter_context(tc.tile_pool(name="osbp", bufs=2))

    # --- constants (Act queue keeps SP free for weights/x) ---
    def chan_const(t_dram, n):
        t = consts.tile([128, NE, n], F32, tag=t_dram.name, name=t_dram.name)
        nc.scalar.dma_start(t[:], t_dram.ap().rearrange("(m p) n -> p m n", p=128))
        return t

    wc_sb = chan_const(wc, KC)
    bc_sb = chan_const(b_conv, 1)
    nch_sb = chan_const(neg_ch, 1)
    bfh_sb = chan_const(b_fh, 1)
    bih_sb = chan_const(b_ih, 1)
    mc_sb = consts.tile([128, 1], F32, tag="mc")
    nc.scalar.dma_start(mc_sb[:], mask_c.ap()[:])
    mu_sb = consts.tile([128, 1], F32, tag="mu")
    nc.scalar.dma_start(mu_sb[:], mask_u.ap()[:])
    zeros = consts.tile([128, TT], F32, tag="zeros")
    nc.vector.memset(zeros[:], 0.0)
    c_zero = consts.tile([128, 1], F32, tag="c_zero")
    nc.vector.memset(c_zero[:], 0.0)
    c_qb = consts.tile([128, 1], F32, tag="c_qb")
    nc.vector.memset(c_qb[:], 0.25000025)
    hc = consts.tile([128, NE], F32, tag="hc")
    plc = consts.tile([128, NE], F32, tag="plc")
    contrib = consts.tile([128, NE], F32, tag="contrib")
    craw = consts.tile([128, NE], F32, tag="craw")
    carry = consts.tile([128, NE], F32, tag="carry")

    h_dram = dram.tile([NTT, 128, NE, TT], BF16, tag="h_spill")
    p_dram = dram.tile([NTT, 128, NE, TT], BF16, tag="p_spill")
    cc_in = dram.tile([E], F32, tag="cc_in")
    cc_out = dram.tile([E], F32, tag="cc_out")

    # --- weights (persistent bf16; w_out shares the w_in_x slot) ---
    wx_sb = wpool.tile([128, NK, E], BF16, tag="wxo", name="wx_sb")
    wg_sb = wpool.tile([128, NK, 2 * E], BF16, tag="wg", name="wg_sb")
    wgi_sb = wpool.tile([128, NK, E], BF16, tag="wgi", name="wgi_sb")
    wx_src = w_in_x.ap().rearrange("(k p) e -> p k e", p=128)
    wg_src = w_gates.ap().rearrange("(k p) f -> p k f", p=128)
    wgi_src = w_in_g.ap().rearrange("(k p) e -> p k e", p=128)
    wo_src = w_out.ap().rearrange("(k p) c -> p k c", p=128)
    xT_r = xT.ap().rearrange("(k p) t -> p k t", p=128)
    halo_r = xa_halo.ap().rearrange("(m p) n -> p m n", p=128)

    def load_x_tile(tt):
        t = xs.tile([128, NK, TT], BF16, tag="xstream", name="xt")
        nc.sync.dma_start(t[:], xT_r[:, :, tt * TT:(tt + 1) * TT])
        return t

    # ================= pass 1: xa proj + conv + gates ==================
    # The u/scan stage for tile tt runs one tile deferred (during tt+1)
    # so the next tile's conv is first in the DVE queue and PE never
    # waits on the scan tail.
    def deferred_act(st):
        tt, sfs, sis, xcs = st
        als, bes = {}, {}
        for m in range(NE):  # alpha = exp(-c/2*vf - c/2)  [same table as tanh]
            al = alp.tile([128, TT], F32, tag="al", name="al")
            nc.scalar.activation(al[:], sfs[m][:], AF.Exp,
                                 scale=nch_sb[:, m, 0:1],
                                 bias=nch_sb[:, m, 0:1])
            als[m] = al
            a2 = a2p.tile([128, TT], F32, tag="a2", name="a2")
            nc.gpsimd.tensor_mul(a2[:], al[:], al[:])
            bes[m] = a2
        for m in range(NE):  # beta/2 = sqrt(0.25000025 - 0.25*alpha^2)
            be = bep.tile([128, TT], F32, tag="be", name="be")
            nc.scalar.activation(be[:], bes[m][:], AF.Sqrt,
                                 scale=-0.25, bias=c_qb[:])
            bes[m] = be
        return als, bes

    def deferred_dve(st, als, bes):
        tt, sfs, sis, xcs = st
        h_all = spl.tile([128, NE, TT], BF16, tag="hall", name="h_all")
        p_all = spl.tile([128, NE, TT], BF16, tag="pall", name="p_all")
        for m in range(NE):
            bs = bsp.tile([128, TT], F32, tag="bs", name="bs")
            nc.vector.scalar_tensor_tensor(bs[:], sis[m][:], 1.0, bes[m][:],
                                           op0=OP.add, op1=OP.mult)
            u = up.tile([128, TT], F32, tag="u", name="u")
            nc.vector.tensor_mul(u[:], bs[:], xcs[m][:])
            nc.vector.tensor_tensor_scan(
                h_all[:, m], als[m][:], u[:],
                0.0 if tt == 0 else hc[:, m:m + 1],
                op0=OP.mult, op1=OP.add)
            nc.vector.tensor_copy(hc[:, m:m + 1], h_all[:, m, TT - 1:TT])
            nc.vector.tensor_tensor_scan(
                p_all[:, m], als[m][:], zeros[:],
                1.0 if tt == 0 else plc[:, m:m + 1],
                op0=OP.mult, op1=OP.add)
            nc.vector.tensor_copy(plc[:, m:m + 1], p_all[:, m, TT - 1:TT])
        nc.gpsimd.dma_start(h_dram[tt], h_all[:])
        nc.gpsimd.dma_start(p_dram[tt], p_all[:])

    prev_xa = None
    pending = None
    xt_p2 = None
    for tt in range(NTT):
        if tt == 0:
            xt = xs.tile([128, NK, TT], BF16, tag="xstream", name="xt")
            for k in range(NK):
                nc.sync.dma_start(wx_sb[:, k], wx_src[:, k])
                nc.sync.dma_start(xt[:, k], xT_r[:, k, 0:TT])
            for k in range(NK):
                nc.sync.dma_start(wg_sb[:, k], wg_src[:, k])
        else:
            xt = load_x_tile(tt)

        xas, xcs = [], []
        for m in range(NE):
            pa = ps.tile([128, TT], F32, tag="ps", name="pa")
            for k in range(NK):
                nc.tensor.matmul(pa[:], wx_sb[:, k, m * 128:(m + 1) * 128],
                                 xt[:, k], start=(k == 0), stop=(k == NK - 1))
            xa = xap.tile([128, TT + KC - 1], BF16, tag=f"xa{m}", name="xa")
            nc.vector.tensor_copy(xa[:, KC - 1:TT + KC - 1], pa[:])
            if tt == 0:
                nc.scalar.dma_start(xa[:, 0:KC - 1], halo_r[:, m])
            else:
                nc.vector.tensor_copy(xa[:, 0:KC - 1],
                                      prev_xa[m][:, TT:TT + KC - 1])
            xc = xcp.tile([128, TT], BF16, tag=f"xc{m}", name="xc")
            nc.vector.tensor_scalar(
                xc[:], xa[:, 0:TT], wc_sb[:, m, 0:1], bc_sb[:, m, 0:1],
                op0=OP.mult, op1=OP.add)
            for j in range(1, KC):
                nc.vector.scalar_tensor_tensor(
                    xc[:], xa[:, j:j + TT], wc_sb[:, m, j:j + 1],
                    xc[:], op0=OP.mult, op1=OP.add)
            xas.append(xa)
            xcs.append(xc)
        if pending is not None:
            d_als, d_bes = deferred_act(pending)
            deferred_dve(pending, d_als, d_bes)

        sfs, sis = {}, {}
        for g in range(2):
            ms = range(g * 4, g * 4 + 4)
            pfs, pis = {}, {}
            for m in ms:
                pf = ps.tile([128, TT], F32, tag="ps", name="pf")
                for k in range(NK):
                    nc.tensor.matmul(pf[:], wg_sb[:, k, m * 128:(m + 1) * 128],
                                     xcs[k][:], start=(k == 0), stop=(k == NK - 1))
                pfs[m] = pf
                pi = ps.tile([128, TT], F32, tag="ps", name="pi")
                for k in range(NK):
                    nc.tensor.matmul(pi[:], wg_sb[:, k, E + m * 128:E + (m + 1) * 128],
                                     xcs[k][:], start=(k == 0), stop=(k == NK - 1))
                pis[m] = pi
            for m in ms:  # sigmoid(x) = 0.5*tanh(x/2)+0.5, folded downstream
                sf = sfp.tile([128, TT], BF16, tag="sf", name="sf")
                nc.scalar.activation(sf[:], pfs[m][:], AF.Tanh,
                                     scale=0.5, bias=bfh_sb[:, m, 0:1])
                sfs[m] = sf
                si = sip.tile([128, TT], BF16, tag="si", name="si")
                nc.scalar.activation(si[:], pis[m][:], AF.Tanh,
                                     scale=0.5, bias=bih_sb[:, m, 0:1])
                sis[m] = si

        for k in (2 * tt, 2 * tt + 1):
            nc.sync.dma_start(wgi_sb[:, k], wgi_src[:, k])
        prev_xa = xas
        pending = (tt, sfs, sis, xcs)
        if tt == NTT - 1:
            xt_p2 = load_x_tile(0)

    d_als, d_bes = deferred_act(pending)
    deferred_dve(pending, d_als, d_bes)
    pending = None

    # w_out loads into the (now dead) w_in_x slot
    wo_sb = wpool.tile([128, NK, DIM], BF16, tag="wxo", name="wo_sb")
    for k in range(NK):
        nc.sync.dma_start(wo_sb[:, k], wo_src[:, k])

    # ====== pass 2 prefill: first two gate projections (carry-free) ====
    def gate_phase(tt, xt):
        h2 = spl.tile([128, NE, TT], BF16, tag="hall", name="h2")
        nc.sync.dma_start(h2[:], h_dram[tt])
        p2 = spl.tile([128, NE, TT], BF16, tag="pall", name="p2")
        nc.sync.dma_start(p2[:], p_dram[tt])
        ggs = []
        for m in range(NE):
            pg = ps.tile([128, TT], F32, tag="ps", name="pg")
            for k in range(NK):
                nc.tensor.matmul(pg[:], wgi_sb[:, k, m * 128:(m + 1) * 128],
                                 xt[:, k], start=(k == 0), stop=(k == NK - 1))
            gg = xcp.tile([128, TT], BF16, tag=f"xc{m}", name="gg")
            nc.scalar.activation(gg[:], pg[:], AF.Gelu, bias=c_zero[:])
            ggs.append(gg)
        return h2, p2, ggs

    def y_phase(tt, st):
        h2, p2, ggs = st
        yt = []
        for m in range(NE):
            hv = hvp.tile([128, TT], F32, tag="hv", name="hv")
            nc.vector.scalar_tensor_tensor(
                hv[:], p2[:, m], carry[:, m:m + 1], h2[:, m],
                op0=OP.mult, op1=OP.add)
            y = xap.tile([128, TT + KC - 1], BF16, tag=f"xa{m}", name="y")
            nc.vector.tensor_mul(y[:, 0:TT], ggs[m][:], hv[:])
            yt.append(y)
        return yt

    def out_phase(tt, yt):
        for q in range(TT // 128):
            pos = [ps.tile([128, 512], F32, tag="ps", name="po")
                   for _ in range(2)]
            for k in range(NE):
                for n in range(2):
                    nc.tensor.matmul(
                        pos[n][:], yt[k][:, q * 128:(q + 1) * 128],
                        wo_sb[:, k, n * 512:(n + 1) * 512],
                        start=(k == 0), stop=(k == NE - 1))
            osb = osbp.tile([128, DIM], BF16, tag="osb", name="osb")
            for n in range(2):
                nc.scalar.copy(osb[:, n * 512:(n + 1) * 512], pos[n][:])
            nc.scalar.dma_start(
                out.ap()[tt * TT + q * 128:tt * TT + (q + 1) * 128, :],
                osb[:])

    gstates = {}
    gstates[0] = gate_phase(0, xt_p2)
    gstates[1] = gate_phase(1, load_x_tile(1))

    # ================= carry exchange (pairwise AllReduce, 4KB) ========
    nc.vector.tensor_scalar(contrib[:], hc[:], mc_sb[:, 0:1], None,
                            op0=OP.mult)
    nc.sync.dma_start(cc_in[:].rearrange("(j p) -> p j", p=128), contrib[:])
    if profile_mode:
        nc.sync.dma_start(cc_out[:], cc_in[:])
    else:
        nc.gpsimd.collective_compute(
            "AllReduce", OP.add,
            replica_groups=[[0, 1], [2, 3], [4, 5], [6, 7]],
            ins=[cc_in[:].opt()], outs=[cc_out[:].opt()])
    nc.sync.dma_start(craw[:], cc_out[:].rearrange("(j p) -> p j", p=128))
    nc.vector.tensor_scalar(carry[:], craw[:], mu_sb[:, 0:1], None,
                            op0=OP.mult)

    # ================= pass 2: correction + out proj ===================
    for tt in range(NTT):
        yt = y_phase(tt, gstates.pop(tt))
        if tt + 2 < NTT:
            gstates[tt + 2] = gate_phase(tt + 2, load_x_tile(tt + 2))
        out_phase(tt, yt)


_NC_CACHE = {}


def _get_nc():
    if "nc" not in _NC_CACHE:
        _NC_CACHE["nc"] = _build_kernel()
    return _NC_CACHE["nc"]


def _softplus(x):
    return np.logaddexp(0.0, x)


def kernel(x, w_in, w_conv, b_conv, w_gates, b_gates, forget_base, w_out,
           _want_trace=False):
    BF = ml_dtypes.bfloat16
    x = np.asarray(x, dtype=np.float32)
    w_in = np.asarray(w_in, dtype=np.float32)
    w_conv = np.asarray(w_conv, dtype=np.float32)
    b_conv = np.asarray(b_conv, dtype=np.float32)
    w_gates = np.asarray(w_gates, dtype=np.float32)
    b_gates = np.asarray(b_gates, dtype=np.float32)
    forget_base = np.asarray(forget_base, dtype=np.float32)
    w_out = np.asarray(w_out, dtype=np.float32)

    nc = _get_nc()

    w_in_g = np.ascontiguousarray(w_in[:E].T).astype(BF)     # [DIM, E]
    w_in_x = np.ascontiguousarray(w_in[E:].T).astype(BF)     # [DIM, E]
    w_gates_T = np.ascontiguousarray(w_gates.T).astype(BF)   # [E, 2E]
    w_out_T = np.ascontiguousarray(w_out.T).astype(BF)       # [E, DIM]
    wc_r = np.ascontiguousarray(w_conv.reshape(E, KC))
    neg_c = (-8.0 * _softplus(forget_base.astype(np.float64))).astype(
        np.float32)[:, None]

    common = {
        "w_in_g": w_in_g, "w_in_x": w_in_x, "w_gates": w_gates_T,
        "w_out": w_out_T, "wc": wc_r, "b_conv": b_conv[:, None].copy(),
        "neg_ch": 0.5 * neg_c,
        "b_fh": 0.5 * b_gates[:E, None], "b_ih": 0.5 * b_gates[E:, None],
    }
    in_maps = []
    for k in range(N_CORES):
        b, half = k // 2, k % 2
        t0 = half * T_LOC
        xT_loc = np.ascontiguousarray(x[b, t0:t0 + T_LOC, :].T).astype(BF)
        if half == 1:
            xa_halo = (x[b, t0 - (KC - 1):t0, :] @ w_in[E:].T).T
            xa_halo = np.ascontiguousarray(xa_halo).astype(BF)
        else:
            xa_halo = np.zeros((E, KC - 1), dtype=BF)
        mc = np.full((128, 1), 1.0 if half == 0 else 0.0, dtype=np.float32)
        mu = np.full((128, 1), 0.0 if half == 0 else 1.0, dtype=np.float32)
        in_maps.append({**common, "xT": xT_loc, "xa_halo": xa_halo,
                        "mask_c": mc, "mask_u": mu})

    res = run_bass_kernel_spmd(nc, in_maps, core_ids=list(range(N_CORES)),
                               trace=_want_trace)
    out_full = np.empty((B, T, DIM), dtype=np.float32)
    for k in range(N_CORES):
        b, half = k // 2, k % 2
        out_full[b, half * T_LOC:(half + 1) * T_LOC, :] = \
            res.results[k]["out"].astype(np.float32)
    if _want_trace:
        return out_full, res
    return out_full


# revision 16
# speedup vs baseline: 1.0726x; 1.0726x over previous
"""Hawk (RG-LRU) block kernel for Trainium2, SPMD over 8 NeuronCores.

Sharding: tokens. Core k handles batch b=k//2, half h=k%2 (2048 tokens).
Weights replicated, host-transposed, bf16 (full PE rate, half the HBM
traffic). Two fused passes over 4 token tiles of 512:

  pass 1: in-proj -> causal conv (DVE, bf16) -> gates matmul ->
          tanh/exp/ln activation chain (sigmoid via tanh so tanh+exp
          share one act-func table; beta via ln+exp) -> u ->
          h-scan + alpha-prefix-scan (DVE, fp32 state, bf16 out);
          h,p spill bf16 via the idle GPSIMD DMA queue.
  carry:  pairwise 4KB AllReduce moves the cross-half scan carry.
  pass 2: gate-proj + gelu, carry correction, out-proj one tile behind
          so PE never waits on the vector chain; out stored bf16.

alpha^2 runs on the (otherwise idle) GPSIMD engine. DMAs are batched
(one per tile per stream) and spread over SP/Act/Pool queues to avoid
sequencer head-of-line blocking.
"""
import sys

sys.path.insert(0, "/opt/trn_rl_repo")

import numpy as np
import ml_dtypes
from contextlib import ExitStack

import concourse.bass as bass
import concourse.tile as tile
import concourse.bacc as bacc
from concourse import mybir
from concourse.bass_utils import run_bass_kernel_spmd

F32 = mybir.dt.float32
BF16 = mybir.dt.bfloat16
AF = mybir.ActivationFunctionType
OP = mybir.AluOpType

B, T, DIM = 4, 4096, 1024
E = 1024
KC = 4
N_CORES = 8
T_LOC = T // 2
TT = 512
NTT = T_LOC // TT   # 4
NE = E // 128       # 8
NK = DIM // 128     # 8

ALPHA2_ON_POOL = True


def _build_kernel(profile_mode=False):
    nc = bacc.Bacc("TRN2", target_bir_lowering=False, debug=False,
                   num_devices=1 if profile_mode else N_CORES)

    xT = nc.dram_tensor("xT", [DIM, T_LOC], BF16, kind="ExternalInput")
    xa_halo = nc.dram_tensor("xa_halo", [E, KC - 1], BF16, kind="ExternalInput")
    w_in_g = nc.dram_tensor("w_in_g", [DIM, E], BF16, kind="ExternalInput")
    w_in_x = nc.dram_tensor("w_in_x", [DIM, E], BF16, kind="ExternalInput")
    w_gates = nc.dram_tensor("w_gates", [E, 2 * E], BF16, kind="ExternalInput")
    w_out = nc.dram_tensor("w_out", [E, DIM], BF16, kind="ExternalInput")
    wc = nc.dram_tensor("wc", [E, KC], F32, kind="ExternalInput")
    b_conv = nc.dram_tensor("b_conv", [E, 1], F32, kind="ExternalInput")
    neg_ch = nc.dram_tensor("neg_ch", [E, 1], F32, kind="ExternalInput")
    b_fh = nc.dram_tensor("b_fh", [E, 1], F32, kind="ExternalInput")
    b_ih = nc.dram_tensor("b_ih", [E, 1], F32, kind="ExternalInput")
    mask_c = nc.dram_tensor("mask_c", [128, 1], F32, kind="ExternalInput")
    mask_u = nc.dram_tensor("mask_u", [128, 1], F32, kind="ExternalInput")
    out = nc.dram_tensor("out", [T_LOC, DIM], BF16, kind="ExternalOutput")

    with tile.TileContext(nc) as tc, ExitStack() as ctx:
        _body(ctx, tc, nc, profile_mode=profile_mode,
              xT=xT, xa_halo=xa_halo, w_in_g=w_in_g, w_in_x=w_in_x,
              w_gates=w_gates, w_out=w_out, wc=wc, b_conv=b_conv,
              neg_ch=neg_ch, b_fh=b_fh, b_ih=b_ih,
              mask_c=mask_c, mask_u=mask_u, out=out)
    nc.compile()
    return nc


def _body(ctx, tc, nc, *, xT, xa_halo, w_in_g, w_in_x, w_gates, w_out, wc,
          b_conv, neg_ch, b_fh, b_ih, mask_c, mask_u, out,
          profile_mode=False):
    consts = ctx.enter_context(tc.tile_pool(name="consts", bufs=1))
    ps = ctx.enter_context(tc.tile_pool(name="ps", bufs=8, space="PSUM"))
    dram = ctx.enter_context(tc.tile_pool(name="dram", bufs=1, space="DRAM"))
    wpool = ctx.enter_context(tc.tile_pool(name="weights", bufs=1, side="right"))
    xs = ctx.enter_context(tc.tile_pool(name="xs", bufs=3))
    xap = ctx.enter_context(tc.tile_pool(name="xap", bufs=2))
    xcp = ctx.enter_context(tc.tile_pool(name="xcp", bufs=2))
    sfp = ctx.enter_context(tc.tile_pool(name="sfp", bufs=9))
    sip = ctx.enter_context(tc.tile_pool(name="sip", bufs=9))
    alp = ctx.enter_context(tc.tile_pool(name="alp", bufs=3))
    a2p = ctx.enter_context(tc.tile_pool(name="a2p", bufs=3))
    bep = ctx.enter_context(tc.tile_pool(name="bep", bufs=3))
    bsp = ctx.enter_context(tc.tile_pool(name="bsp", bufs=2))
    up = ctx.enter_context(tc.tile_pool(name="up", bufs=2))
    spl = ctx.enter_context(tc.tile_pool(name="spl", bufs=2))
    hvp = ctx.enter_context(tc.tile_pool(name="hvp", bufs=2))
    osbp = ctx.enter_context(tc.tile_pool(name="osbp", bufs=2))

    # --- constants (Act queue keeps SP free for weights/x) ---
    def chan_const(t_dram, n):
        t = consts.tile([128, NE, n], F32, tag=t_dram.name, name=t_dram.name)
        nc.scalar.dma_start(t[:], t_dram.ap().rearrange("(m p) n -> p m n", p=128))
        return t

    wc_sb = chan_const(wc, KC)
    bc_sb = chan_const(b_conv, 1)
    nch_sb = chan_const(neg_ch, 1)
    bfh_sb = chan_const(b_fh, 1)
    bih_sb = chan_const(b_ih, 1)
    mc_sb = consts.tile([128, 1], F32, tag="mc")
    nc.scalar.dma_start(mc_sb[:], mask_c.ap()[:])
    mu_sb = consts.tile([128, 1], F32, tag="mu")
    nc.scalar.dma_start(mu_sb[:], mask_u.ap()[:])
    zeros = consts.tile([128, TT], F32, tag="zeros")
    nc.vector.memset(zeros[:], 0.0)
    c_zero = consts.tile([128, 1], F32, tag="c_zero")
    nc.vector.memset(c_zero[:], 0.0)
    c_qb = consts.tile([128, 1], F32, tag="c_qb")
    nc.vector.memset(c_qb[:], 0.25000025)
    hc = consts.tile([128, NE], F32, tag="hc")
    plc = consts.tile([128, NE], F32, tag="plc")
    contrib = consts.tile([128, NE], F32, tag="contrib")
    craw = consts.tile([128, NE], F32, tag="craw")
    carry = consts.tile([128, NE], F32, tag="carry")

    h_dram = dram.tile([NTT, 128, NE, TT], BF16, tag="h_spill")
    p_dram = dram.tile([NTT, 128, NE, TT], BF16, tag="p_spill")
    cc_in = dram.tile([E], F32, tag="cc_in")
    cc_out = dram.tile([E], F32, tag="cc_out")

    # --- weights (persistent bf16; w_out shares the w_in_x slot) ---
    wx_sb = wpool.tile([128, NK, E], BF16, tag="wxo", name="wx_sb")
    wg_sb = wpool.tile([128, NK, 2 * E], BF16, tag="wg", name="wg_sb")
    wgi_sb = wpool.tile([128, NK, E], BF16, tag="wgi", name="wgi_sb")
    wx_src = w_in_x.ap().rearrange("(k p) e -> p k e", p=128)
    wg_src = w_gates.ap().rearrange("(k p) f -> p k f", p=128)
    wgi_src = w_in_g.ap().rearrange("(k p) e -> p k e", p=128)
    wo_src = w_out.ap().rearrange("(k p) c -> p k c", p=128)
    xT_r = xT.ap().rearrange("(k p) t -> p k t", p=128)
    halo_r = xa_halo.ap().rearrange("(m p) n -> p m n", p=128)

    def load_x_tile(tt):
        t = xs.tile([128, NK, TT], BF16, tag="xstream", name="xt")
        nc.sync.dma_start(t[:], xT_r[:, :, tt * TT:(tt + 1) * TT])
        return t

    # ================= pass 1: xa proj + conv + gates ==================
    # The u/scan stage for tile tt runs one tile deferred (during tt+1)
    # so the next tile's conv is first in the DVE queue and PE never
    # waits on the scan tail.
    def deferred_act(st):
        tt, sfs, sis, xcs = st
        als, bes = {}, {}
        for m in range(NE):  # alpha = exp(-c/2*vf - c/2)  [same table as tanh]
            al = alp.tile([128, TT], F32, tag="al", name="al")
            nc.scalar.activation(al[:], sfs[m][:], AF.Exp,
                                 scale=nch_sb[:, m, 0:1],
                                 bias=nch_sb[:, m, 0:1])
            als[m] = al
            a2 = a2p.tile([128, TT], F32, tag="a2", name="a2")
            nc.gpsimd.tensor_mul(a2[:], al[:], al[:])
            bes[m] = a2
        for m in range(NE):  # beta/2 = sqrt(0.25000025 - 0.25*alpha^2)
            be = bep.tile([128, TT], F32, tag="be", name="be")
            nc.scalar.activation(be[:], bes[m][:], AF.Sqrt,
                                 scale=-0.25, bias=c_qb[:])
            bes[m] = be
        return als, bes

    def deferred_dve(st, als, bes):
        tt, sfs, sis, xcs = st
        h_all = spl.tile([128, NE, TT], BF16, tag="hall", name="h_all")
        p_all = spl.tile([128, NE, TT], BF16, tag="pall", name="p_all")
        for m in range(NE):
            bs = bsp.tile([128, TT], F32, tag="bs", name="bs")
            nc.vector.scalar_tensor_tensor(bs[:], sis[m][:], 1.0, bes[m][:],
                                           op0=OP.add, op1=OP.mult)
            u = up.tile([128, TT], F32, tag="u", name="u")
            nc.vector.tensor_mul(u[:], bs[:], xcs[m][:])
            nc.vector.tensor_tensor_scan(
                h_all[:, m], als[m][:], u[:],
                0.0 if tt == 0 else hc[:, m:m + 1],
                op0=OP.mult, op1=OP.add)
            nc.vector.tensor_copy(hc[:, m:m + 1], h_all[:, m, TT - 1:TT])
            nc.vector.tensor_tensor_scan(
                p_all[:, m], als[m][:], zeros[:],
                1.0 if tt == 0 else plc[:, m:m + 1],
                op0=OP.mult, op1=OP.add)
            nc.vector.tensor_copy(plc[:, m:m + 1], p_all[:, m, TT - 1:TT])
        nc.gpsimd.dma_start(h_dram[tt], h_all[:])
        nc.gpsimd.dma_start(p_dram[tt], p_all[:])

    prev_xa = None
    pending = None
    xt_p2 = None
    for tt in range(NTT):
        if tt == 0:
            xt = xs.tile([128, NK, TT], BF16, tag="xstream", name="xt")
            for k in range(NK):
                nc.sync.dma_start(wx_sb[:, k], wx_src[:, k])
                nc.sync.dma_start(xt[:, k], xT_r[:, k, 0:TT])
            for k in range(NK):
                nc.sync.dma_start(wg_sb[:, k], wg_src[:, k])
        else:
            xt = load_x_tile(tt)

        xas, xcs = [], []
        for m in range(NE):
            pa = ps.tile([128, TT], F32, tag="ps", name="pa")
            for k in range(NK):
                nc.tensor.matmul(pa[:], wx_sb[:, k, m * 128:(m + 1) * 128],
                                 xt[:, k], start=(k == 0), stop=(k == NK - 1))
            xa = xap.tile([128, TT + KC - 1], BF16, tag=f"xa{m}", name="xa")
            nc.scalar.copy(xa[:, KC - 1:TT + KC - 1], pa[:])
            if tt == 0:
                nc.scalar.dma_start(xa[:, 0:KC - 1], halo_r[:, m])
            else:
                nc.vector.tensor_copy(xa[:, 0:KC - 1],
                                      prev_xa[m][:, TT:TT + KC - 1])
            xc = xcp.tile([128, TT], BF16, tag=f"xc{m}", name="xc")
            nc.vector.tensor_scalar(
                xc[:], xa[:, 0:TT], wc_sb[:, m, 0:1], bc_sb[:, m, 0:1],
                op0=OP.mult, op1=OP.add)
            for j in range(1, KC):
                nc.vector.scalar_tensor_tensor(
                    xc[:], xa[:, j:j + TT], wc_sb[:, m, j:j + 1],
                    xc[:], op0=OP.mult, op1=OP.add)
            xas.append(xa)
            xcs.append(xc)
        if pending is not None:
            d_als, d_bes = deferred_act(pending)
            deferred_dve(pending, d_als, d_bes)

        sfs, sis = {}, {}
        for g in range(2):
            ms = range(g * 4, g * 4 + 4)
            pfs, pis = {}, {}
            for m in ms:
                pf = ps.tile([128, TT], F32, tag="ps", name="pf")
                for k in range(NK):
                    nc.tensor.matmul(pf[:], wg_sb[:, k, m * 128:(m + 1) * 128],
                                     xcs[k][:], start=(k == 0), stop=(k == NK - 1))
                pfs[m] = pf
                pi = ps.tile([128, TT], F32, tag="ps", name="pi")
                for k in range(NK):
                    nc.tensor.matmul(pi[:], wg_sb[:, k, E + m * 128:E + (m + 1) * 128],
                                     xcs[k][:], start=(k == 0), stop=(k == NK - 1))
                pis[m] = pi
            for m in ms:  # sigmoid(x) = 0.5*tanh(x/2)+0.5, folded downstream
                sf = sfp.tile([128, TT], BF16, tag="sf", name="sf")
                nc.scalar.activation(sf[:], pfs[m][:], AF.Tanh,
                                     scale=0.5, bias=bfh_sb[:, m, 0:1])
                sfs[m] = sf
                si = sip.tile([128, TT], BF16, tag="si", name="si")
                nc.scalar.activation(si[:], pis[m][:], AF.Tanh,
                                     scale=0.5, bias=bih_sb[:, m, 0:1])
                sis[m] = si

        for k in (2 * tt, 2 * tt + 1):
            nc.sync.dma_start(wgi_sb[:, k], wgi_src[:, k])
        prev_xa = xas
        pending = (tt, sfs, sis, xcs)
        if tt == NTT - 1:
            xt_p2 = load_x_tile(0)

    d_als, d_bes = deferred_act(pending)
    deferred_dve(pending, d_als, d_bes)
    pending = None

    # w_out loads into the (now dead) w_in_x slot
    wo_sb = wpool.tile([128, NK, DIM], BF16, tag="wxo", name="wo_sb")
    for k in range(NK):
        nc.sync.dma_start(wo_sb[:, k], wo_src[:, k])

    # ====== pass 2 prefill: first two gate projections (carry-free) ====
    def gate_phase(tt, xt):
        h2 = spl.tile([128, NE, TT], BF16, tag="hall", name="h2")
        nc.sync.dma_start(h2[:], h_dram[tt])
        p2 = spl.tile([128, NE, TT], BF16, tag="pall", name="p2")
        nc.sync.dma_start(p2[:], p_dram[tt])
        ggs = []
        for m in range(NE):
            pg = ps.tile([128, TT], F32, tag="ps", name="pg")
            for k in range(NK):
                nc.tensor.matmul(pg[:], wgi_sb[:, k, m * 128:(m + 1) * 128],
                                 xt[:, k], start=(k == 0), stop=(k == NK - 1))
            gg = xcp.tile([128, TT], BF16, tag=f"xc{m}", name="gg")
            nc.scalar.activation(gg[:], pg[:], AF.Gelu, bias=c_zero[:])
            ggs.append(gg)
        return h2, p2, ggs

    def y_phase(tt, st):
        h2, p2, ggs = st
        yt = []
        for m in range(NE):
            hv = hvp.tile([128, TT], F32, tag="hv", name="hv")
            nc.vector.scalar_tensor_tensor(
                hv[:], p2[:, m], carry[:, m:m + 1], h2[:, m],
                op0=OP.mult, op1=OP.add)
            y = xap.tile([128, TT + KC - 1], BF16, tag=f"xa{m}", name="y")
            nc.vector.tensor_mul(y[:, 0:TT], ggs[m][:], hv[:])
            yt.append(y)
        return yt

    def out_phase(tt, yt):
        for q in range(TT // 128):
            pos = [ps.tile([128, 512], F32, tag="ps", name="po")
                   for _ in range(2)]
            for k in range(NE):
                for n in range(2):
                    nc.tensor.matmul(
                        pos[n][:], yt[k][:, q * 128:(q + 1) * 128],
                        wo_sb[:, k, n * 512:(n + 1) * 512],
                        start=(k == 0), stop=(k == NE - 1))
            osb = osbp.tile([128, DIM], BF16, tag="osb", name="osb")
            for n in range(2):
                nc.scalar.copy(osb[:, n * 512:(n + 1) * 512], pos[n][:])
            nc.scalar.dma_start(
                out.ap()[tt * TT + q * 128:tt * TT + (q + 1) * 128, :],
                osb[:])

    gstates = {}
    gstates[0] = gate_phase(0, xt_p2)
    gstates[1] = gate_phase(1, load_x_tile(1))

    # ================= carry exchange (pairwise AllReduce, 4KB) ========
    nc.vector.tensor_scalar(contrib[:], hc[:], mc_sb[:, 0:1], None,
                            op0=OP.mult)
    nc.sync.dma_start(cc_in[:].rearrange("(j p) -> p j", p=128), contrib[:])
    if profile_mode:
        nc.sync.dma_start(cc_out[:], cc_in[:])
    else:
        nc.gpsimd.collective_compute(
            "AllReduce", OP.add,
            replica_groups=[[0, 1], [2, 3], [4, 5], [6, 7]],
            ins=[cc_in[:].opt()], outs=[cc_out[:].opt()])
    nc.sync.dma_start(craw[:], cc_out[:].rearrange("(j p) -> p j", p=128))
    nc.vector.tensor_scalar(carry[:], craw[:], mu_sb[:, 0:1], None,
                            op0=OP.mult)

    # ================= pass 2: correction + out proj ===================
    for tt in range(NTT):
        yt = y_phase(tt, gstates.pop(tt))
        if tt + 2 < NTT:
            gstates[tt + 2] = gate_phase(tt + 2, load_x_tile(tt + 2))
        out_phase(tt, yt)


_NC_CACHE = {}


def _get_nc():
    if "nc" not in _NC_CACHE:
        _NC_CACHE["nc"] = _build_kernel()
    return _NC_CACHE["nc"]


def _softplus(x):
    return np.logaddexp(0.0, x)


def kernel(x, w_in, w_conv, b_conv, w_gates, b_gates, forget_base, w_out,
           _want_trace=False):
    BF = ml_dtypes.bfloat16
    x = np.asarray(x, dtype=np.float32)
    w_in = np.asarray(w_in, dtype=np.float32)
    w_conv = np.asarray(w_conv, dtype=np.float32)
    b_conv = np.asarray(b_conv, dtype=np.float32)
    w_gates = np.asarray(w_gates, dtype=np.float32)
    b_gates = np.asarray(b_gates, dtype=np.float32)
    forget_base = np.asarray(forget_base, dtype=np.float32)
    w_out = np.asarray(w_out, dtype=np.float32)

    nc = _get_nc()

    w_in_g = np.ascontiguousarray(w_in[:E].T).astype(BF)     # [DIM, E]
    w_in_x = np.ascontiguousarray(w_in[E:].T).astype(BF)     # [DIM, E]
    w_gates_T = np.ascontiguousarray(w_gates.T).astype(BF)   # [E, 2E]
    w_out_T = np.ascontiguousarray(w_out.T).astype(BF)       # [E, DIM]
    wc_r = np.ascontiguousarray(w_conv.reshape(E, KC))
    neg_c = (-8.0 * _softplus(forget_base.astype(np.float64))).astype(
        np.float32)[:, None]

    common = {
        "w_in_g": w_in_g, "w_in_x": w_in_x, "w_gates": w_gates_T,
        "w_out": w_out_T, "wc": wc_r, "b_conv": b_conv[:, None].copy(),
        "neg_ch": 0.5 * neg_c,
        "b_fh": 0.5 * b_gates[:E, None], "b_ih": 0.5 * b_gates[E:, None],
    }
    in_maps = []
    for k in range(N_CORES):
        b, half = k // 2, k % 2
        t0 = half * T_LOC
        xT_loc = np.ascontiguousarray(x[b, t0:t0 + T_LOC, :].T).astype(BF)
        if half == 1:
            xa_halo = (x[b, t0 - (KC - 1):t0, :] @ w_in[E:].T).T
            xa_halo = np.ascontiguousarray(xa_halo).astype(BF)
        else:
            xa_halo = np.zeros((E, KC - 1), dtype=BF)
        mc = np.full((128, 1), 1.0 if half == 0 else 0.0, dtype=np.float32)
        mu = np.full((128, 1), 0.0 if half == 0 else 1.0, dtype=np.float32)
        in_maps.append({**common, "xT": xT_loc, "xa_halo": xa_halo,
                        "mask_c": mc, "mask_u": mu})

    res = run_bass_kernel_spmd(nc, in_maps, core_ids=list(range(N_CORES)),
                               trace=_want_trace)
    out_full = np.empty((B, T, DIM), dtype=np.float32)
    for k in range(N_CORES):
        b, half = k // 2, k % 2
        out_full[b, half * T_LOC:(half + 1) * T_LOC, :] = \
            res.results[k]["out"].astype(np.float32)
    if _want_trace:
        return out_full, res
    return out_full
